# revision 66
# baseline (speedup 1.0000x reference)
"""Multi-head masked self-attention on 8 trn2 NeuronCores.

Problem: B=2, T=2048, H=1024, nH=16 heads (head_dim=64), causal softmax
attention with QKV projections; scores scaled by 1/sqrt(H).

Sharding: heads across cores (2 heads per core), both batches on every core
(B*nH = 32 (b,h) pairs -> 4 per core). QKV weights column-sharded by head:
core m gets W[128m:128m+128, :] of each projection matrix.

The ACT engine (exp eviction of the score stream, 0.833 ns/elem, no fast
mode) is the roofline: ~36 us of exp per batch. The schedule keeps ACT as
gapless as possible, and keeps DMA/instruction counts low for the
hardware's (unmodeled-in-sim) per-descriptor and per-ldweights costs:
  - DMA issue (descriptor gen ~1.2-1.9 us of queue time each) is spread
    over the SP and Pool queues: bulk loads metered through the Pool
    (SWDGE) queue in priority order, DR shuffles and output writebacks on
    SP, the cold-start K shuffle on the idle ACT queue.
  - xt (V-path) loads move in 256-column halves — the narrowest width
    with full DMA bandwidth — so a bulk load never holds the serial DMA
    engines for more than ~1.5 us in front of a critical shuffle copy.
  - Q/K prep runs one projection tile ahead of the V chain, and each
    q-tile's V chain is drained at k-block 1 of its own k-loop, behind
    the first two queued S matmuls, so the PE never parks prep work in
    front of the exp stream.

Attention per q-tile of 512 (4 q-tiles):
  S^T[k, q] = K^T.T(128-slice) @ Q^T -> PSUM f32 [128, 2, 512] (two banks,
      one per head), fp8 DoubleRow, ACT evicts exp(S/32) -> bf16 (no
      row-max: |S/32| << 1 for this distribution), multiplicative bf16
      triangle mask on diagonal blocks (DVE).
  O'^T [65, 512] += V'[kb].T @ P^T[kb], one PSUM bank per head (row 64
      accumulates the softmax denominator Z from the same quantized P),
      trailing the score stream by two k-blocks.
  Finalize is software-pipelined into the NEXT q-tile's k-loop: per head a
      DVE eviction [65, 512] -> SBUF, 4 PE transposes into one PSUM bank,
      one DVE reciprocal, one broadcasted DVE multiply; one merged output
      DMA per q-tile (both heads) on the SP queue. The last q-tile
      finalizes inline with per-head writebacks to shorten the tail.

PSUM budget (8 banks): psS 2x2, psO 1+1, shared ring 2 (projection psA
[128,512] / V' transposes / finalize transposes).
"""
import sys

sys.path.insert(0, "/opt/trn_rl_repo")

import numpy as np

B = 2
T = 2048
H = 1024
NHEADS = 16
HD = 64
NCORES = 8
HEADS_PER_CORE = NHEADS // NCORES  # 2
P = 128
CB = H // P            # 8 contraction blocks for projections
QTILE = 512
NQT = T // QTILE       # 4 q-tiles
NTT = T // QTILE       # 4 projection column tiles
NKB = T // P           # 16 k-blocks
SCALE = 1.0 / np.sqrt(np.float32(H))  # 1/32


def _to_bf16(x: np.ndarray) -> np.ndarray:
    import ml_dtypes

    return np.ascontiguousarray(
        np.asarray(x, np.float32).astype(ml_dtypes.bfloat16)
    )


# DoubleRow channel permutation for the Q/K projections: on-chip channel
# pi feeds DR-layout slot [p, hd] = [pi // 4, pi % 4] via a linear-order
# SBUF->SBUF DMA, and slot [p, (h, dt)] must hold logical head channel
# d = 64h + 32dt + p of the core's 128-channel slice.
_DR_PERM = np.array(
    [64 * ((pi % 4) // 2) + 32 * (pi % 2) + pi // 4 for pi in range(128)]
)


def _build_program(reps: int = 1):
    import contextlib
    import concourse.tile as tile
    from concourse import bacc, mybir
    from concourse.masks import make_identity
    from concourse.bass import ts

    F32 = mybir.dt.float32
    BF16 = mybir.dt.bfloat16
    F8 = mybir.dt.float8e4
    ActF = mybir.ActivationFunctionType
    Alu = mybir.AluOpType

    nc = bacc.Bacc("TRN2", target_bir_lowering=False, debug=False)

    xt_d = nc.dram_tensor("xt", [B, H, T], BF16, kind="ExternalInput")
    x8_d = nc.dram_tensor("x8", [B, P, 2, 4, T], F8, kind="ExternalInput")
    w_d = {"v": nc.dram_tensor("wvt", [H, P], BF16,
                               kind="ExternalInput")}
    w8_d = nc.dram_tensor("w8qk", [2, P, 2, 4, P], F8,
                          kind="ExternalInput")
    b_d = nc.dram_tensor("bqkv", [3, P], F32, kind="ExternalInput")
    out_d = nc.dram_tensor("out", [B, T, P], F32, kind="ExternalOutput")

    with tile.TileContext(nc) as tc:
        with (
            tc.tile_pool(name="const", bufs=1) as const,
            tc.tile_pool(name="xt", bufs=2) as xt_pool,
            tc.tile_pool(name="qkv", bufs=2) as qkv_pool,
            tc.tile_pool(name="qkdr", bufs=2) as qkdr_pool,
            tc.tile_pool(name="vp", bufs=4) as vp_pool,
            tc.tile_pool(name="pt", bufs=5) as pt_pool,
            tc.tile_pool(name="osb", bufs=4) as osb_pool,
            tc.tile_pool(name="fin", bufs=6) as fin_pool,
            tc.tile_pool(name="psmm", bufs=2, space="PSUM") as psmm,
            tc.tile_pool(name="pso", bufs=1, space="PSUM") as pso,
            tc.tile_pool(name="pstr", bufs=2, space="PSUM") as pstr,
        ):
            # ---- constants ----
            ident = const.tile([P, P], BF16, tag="ident")
            make_identity(nc, ident[:])
            # multiplicative causal triangle for the diagonal 128x128 block
            # of a k-block: keep (1) where k_local <= q_local else 0; applied
            # to P after the exp so the mask is off the S->exp critical path
            trimask = const.tile([P, P], BF16, tag="trimask")
            nc.gpsimd.memset(trimask[:], 1.0)
            nc.gpsimd.affine_select(
                out=trimask[:],
                in_=trimask[:],
                compare_op=mybir.AluOpType.is_ge,
                fill=0.0,
                base=0,
                pattern=[[1, P]],
                channel_multiplier=-1,
            )
            ones16 = const.tile([P, NKB], BF16, tag="ones16")
            nc.vector.memset(ones16[:], 1.0)

            w_sb = {"v": const.tile([P, CB, P], BF16, tag="wv", name="wv")}
            w8qk = const.tile([P, 2, 2, 4, P], F8, tag="w8qk", name="w8qk")
            w8_sb = {"q": w8qk[:, 0], "k": w8qk[:, 1]}
            bias3 = const.tile([P, 3], F32, tag="b3", name="b3")
            bias_sb = {n: bias3[:, i : i + 1] for i, n in enumerate("qkv")}

            rep_ctx = (
                tc.For_i(0, reps, 1,
                         hint_engines=(mybir.EngineType.PE,
                                       mybir.EngineType.Activation,
                                       mybir.EngineType.DVE,
                                       mybir.EngineType.SP))
                if reps > 1 else contextlib.nullcontext()
            )
            with rep_ctx:
              # ---------- prep stream ----------
              class Prep:
                  """Generator of ('step', fn) / ('pace', n) /
                  (marker-kind, idx) items with marker memory, so drains
                  are idempotent. ('pace', n) items hold opportunistic
                  pops until global attention progress reaches n, so a
                  batch's projection work is never emitted into the PE
                  queue before its loads can plausibly have landed."""

                  def __init__(self, gen):
                      self.gen = gen
                      self.seen = set()
                      self.pending = None
                      self.done = False

                  def _next(self):
                      if self.pending is not None:
                          it, self.pending = self.pending, None
                          return it
                      it = next(self.gen, None)
                      if it is None:
                          self.done = True
                      return it

                  def drain_until(self, marker):
                      while marker not in self.seen and not self.done:
                          it = self._next()
                          if it is None:
                              return
                          kind, val = it
                          if kind == "step":
                              val()
                          elif kind != "pace":
                              self.seen.add((kind, val))

                  def pop(self, budget, pace):
                      while budget > 0 and not self.done:
                          it = self._next()
                          if it is None:
                              return
                          kind, val = it
                          if kind == "step":
                              val()
                              budget -= 1
                          elif kind == "pace":
                              if val > pace:
                                  self.pending = it
                                  return
                          else:
                              self.seen.add((kind, val))

              def pop_steps(preps, budget, pace):
                  for p in preps:
                      if not p.done:
                          p.pop(budget, pace)
                          return

              def prep_setup(b):
                  """Projection + V'-build for batch b as a stream of small
                  step closures with ('qk', tt) / ('v', kb) markers.

                  DMA queue assignment (descriptor-gen cost ~1.2-1.7 us
                  serializes per queue): x8 quarters + Q-shuffles on SP;
                  weights, K-shuffles and xt chunks on Pool, ordered so the
                  shuffle copies hit the (serial) DMA engines before the
                  bulk xt loads."""
                  qt_sb = qkv_pool.tile([P, T], F8, tag="qt", name="qt_sb")
                  kt_sb = qkv_pool.tile([P, T], F8, tag="kt", name="kt_sb")
                  vt_sb = qkv_pool.tile([P, T], BF16, tag="vt", name="vt_sb")
                  dsts = {"q": qt_sb, "k": kt_sb, "v": vt_sb}
                  qt_dr = qkdr_pool.tile([32, 4, T], F8, tag="qdr",
                                         name="qt_dr")
                  kt_dr = qkdr_pool.tile([32, 4, T], F8, tag="kdr",
                                         name="kt_dr")
                  drs = {"q": qt_dr, "k": kt_dr}
                  state = {}
                  vprime = []
                  xt = xt_pool.tile([P, CB, T], BF16, tag="xt", name="xt_sb")
                  x8 = xt_pool.tile([P, 2, 4, T], F8, tag="x8", name="x8_sb")

                  def load_x8(c, eng=None):
                      def run():
                          (eng or nc.gpsimd).dma_start(
                              x8[:, :, :, ts(c, T // 4)],
                              x8_d[b, :, :, :, ts(c, T // 4)],
                          )
                      return run

                  def load_xt(c0, w):
                      """cols [c0*128, (c0+w)*128), all cb, on Pool."""
                      def run():
                          nc.gpsimd.dma_start(
                              xt[:, :, c0 * P : (c0 + w) * P],
                              xt_d[b, :, c0 * P : (c0 + w) * P]
                              .rearrange("(cb p) t -> p cb t", p=P),
                          )
                      return run

                  prep_out[b] = (qt_dr, kt_dr, vprime)

                  def alloc_psa():
                      state["psA"] = pstr.tile([P, QTILE], F32, tag="tr",
                                               name="psA")

                  def mms(n, tt):
                      def run():
                          # fp8 DoubleRow: contraction c = 256j+128dt+p,
                          # 4 column-pair passes of 256 each
                          for j in range(4):
                              nc.tensor.matmul(
                                  state["psA"][:],
                                  w8_sb[n][:, :, j, :],
                                  x8[:, :, j, ts(tt, QTILE)],
                                  start=(j == 0),
                                  stop=(j == 3),
                                  perf_mode=mybir.MatmulPerfMode.DoubleRow,
                              )
                      return run

                  def evict(n, tt):
                      def run():
                          nc.vector.tensor_scalar_add(
                              dsts[n][:, ts(tt, QTILE)],
                              state["psA"][:],
                              bias_sb[n],
                          )
                      return run

                  def mms_v(c0, w, half):
                      """V projection for cols [c0*128, (c0+w)*128), cb
                      blocks [4*half, 4*half+4)."""
                      def run():
                          for cb in range(4 * half, 4 * half + 4):
                              nc.tensor.matmul(
                                  state["psA"][:, : w * P],
                                  w_sb["v"][:, cb, :],
                                  xt[:, cb, c0 * P : (c0 + w) * P],
                                  start=(cb == 0),
                                  stop=(cb == CB - 1),
                              )
                      return run

                  def evict_v(c0, w):
                      def run():
                          nc.vector.tensor_scalar_add(
                              vt_sb[:, c0 * P : (c0 + w) * P],
                              state["psA"][:, : w * P],
                              bias_sb["v"],
                          )
                      return run

                  def alloc_vp():
                      vp2 = vp_pool.tile([P, NKB, 2, HD + 1], BF16, tag="vp",
                                         name="vp2")
                      nc.vector.tensor_copy(
                          vp2[:, :, :, HD],
                          ones16[:, :, None].broadcast_to((P, NKB, 2)),
                      )
                      vprime.append(vp2)

                  def vtr(kb):
                      def run():
                          trp = pstr.tile([P, P], BF16, tag="tr", name="trp")
                          nc.tensor.transpose(
                              trp[:], vt_sb[:, ts(kb, P)], ident[:],
                          )
                          nc.vector.tensor_copy(
                              vprime[0][:, kb, :, :HD],
                              trp[:].rearrange("p (h d) -> p h d", h=2),
                          )
                      return run

                  def drshuf(n, tt):
                      # cold start only: K's shuffle goes out on the idle
                      # ACT queue so it doesn't serialize behind Q's on
                      # the SP sequencer (held through the sem wait)
                      eng = (nc.scalar if (b == 0 and tt == 0 and n == "k")
                             else nc.sync)
                      def run():
                          eng.dma_start(
                              drs[n][:, :, ts(tt, QTILE)],
                              dsts[n][:, ts(tt, QTILE)],
                          )
                      return run

                  def v_half(c):
                      """V chain for k-blocks c, c+1 (256 columns: the
                      narrowest width that keeps full DMA bandwidth), so
                      V'[kb] lands just in time for the trailing O
                      accumulation. The xt load is yielded separately by
                      the caller to control DMA-queue order."""
                      yield ("step", alloc_psa)
                      yield ("step", mms_v(c, 2, 0))
                      yield ("step", mms_v(c, 2, 1))
                      yield ("step", evict_v(c, 2))
                      for kb in range(c, c + 2):
                          yield ("step", vtr(kb))
                          yield ("v", kb)

                  def qk_block(tt):
                      for n in "qk":
                          yield ("step", alloc_psa)
                          yield ("step", mms(n, tt))
                          yield ("step", evict(n, tt))
                          yield ("step", drshuf(n, tt))
                      yield ("qk", tt)

                  def v_tile(tt):
                      """V chain for tile tt: two half loads (a full
                      1 MB quarter would hog the serial DMA engines for
                      ~3 us and starve the DR-shuffle copies), one
                      full-width projection, four transposes."""
                      yield ("step", load_xt(4 * tt, 2))
                      yield ("step", load_xt(4 * tt + 2, 2))
                      yield ("step", alloc_psa)
                      yield ("step", mms_v(4 * tt, 4, 0))
                      yield ("step", mms_v(4 * tt, 4, 1))
                      yield ("step", evict_v(4 * tt, 4))
                      for kb in range(4 * tt, 4 * tt + 4):
                          yield ("step", vtr(kb))
                          yield ("v", kb)

                  def gen():
                      # Q/K prep runs one tile ahead of the V chain: the
                      # exp stream is gated by Q/K only, and V'[kb] is
                      # needed one k-block behind the exp of the same
                      # q-tile. x8 quarter 0 of batch 0 on SP (the
                      # cold-start critical path); everything else is
                      # metered through the Pool descriptor-gen queue.
                      # Batch 1's pace marks hold its projection work
                      # until batch 0's attention has progressed far
                      # enough for batch 1's loads to have landed.
                      yield ("step", load_x8(0, eng=nc.sync if b == 0
                                             else None))
                      yield ("step", alloc_vp)
                      for it in qk_block(0):
                          yield it
                      yield ("step", load_xt(0, 2))
                      yield ("step", load_x8(1))
                      for it in v_half(0):
                          yield it
                      for it in qk_block(1):
                          yield it
                      yield ("step", load_xt(2, 2))
                      for it in v_half(2):
                          yield it
                      for tt in range(2, NTT):
                          yield ("step", load_x8(tt))
                          for it in qk_block(tt):
                              yield it
                          for it in v_tile(tt - 1):
                              yield it
                      for it in v_tile(NTT - 1):
                          yield it

                  return Prep(gen())

              prep_out = {}

              # deferred finalize state: list of (b, qt, fin, filled-count)
              pending = []

              def finalize_head(fb, fqt, oT_h, h, fin, split):
                  """Transpose + normalize one head of a finished q-tile;
                  with split=True, also write it back on its own DMA."""
                  trp4 = pstr.tile([P, 4, P], BF16, tag="tr", name="trp4")
                  for j in range(4):
                      nc.tensor.transpose(
                          trp4[:, j, : HD + 1],
                          oT_h[:, ts(j, P)],
                          ident[: HD + 1, : HD + 1],
                      )
                  rec = fin_pool.tile([P, 4], F32, tag="rec")
                  nc.vector.reciprocal(rec[:], trp4[:, :, HD])
                  nc.vector.tensor_tensor(
                      fin[:, :, h, :],
                      trp4[:, :, :HD],
                      rec[:, :, None].broadcast_to((P, 4, HD)),
                      op=Alu.mult,
                  )
                  if split:
                      nc.sync.dma_start(
                          out_d[fb, ts(fqt, QTILE), ts(h, HD)]
                          .rearrange("(j p) c -> p j c", p=P),
                          fin[:, :, h, :],
                      )

              def flush_pending():
                  if not pending:
                      return
                  fb, fqt, oT = pending.pop()
                  fin = fin_pool.tile([P, 4, 2, HD], F32, tag="fin")
                  for h in range(2):
                      finalize_head(fb, fqt, oT[:, h, :], h, fin, False)
                  # one merged output DMA per q-tile (both heads), on the
                  # SP queue (the Pool queue meters the bulk loads)
                  nc.sync.dma_start(
                      out_d[fb, ts(fqt, QTILE), :].rearrange(
                          "(j p) c -> p j c", p=P
                      ),
                      fin[:].rearrange("p j h d -> p j (h d)"),
                  )

              def attention(b, own, work):
                  """own: this batch's prep stream (('qk', qt)-gated at each
                  q-tile start, ('v', kb)-gated before each trailing O
                  step); work: prep streams drained opportunistically, two
                  steps per k-block.

                  The P@V' accumulation trails the score stream by one
                  k-block so the PE never waits on the exp: the k-loop body
                  issues S(kb), exp(kb), then O(kb-1)."""
                  qt_dr, kt_dr, vprime = prep_out[b]
                  for qt in range(NQT):
                      own.drain_until(("qk", qt))
                      psO = pso.tile([P, 2, QTILE], F32, tag="o",
                                     name="psO")
                      nkb = 4 * qt + 4
                      pts = {}
                      final = b == B - 1 and qt == NQT - 1

                      def o_step(kb):
                          lo = max(kb - 4 * qt, 0) * P
                          pt = pts.pop(kb)
                          for h in range(2):
                              nc.tensor.matmul(
                                  psO[: HD + 1, h, lo:QTILE],
                                  vprime[0][:, kb, h, :],
                                  pt[:, h, lo:QTILE],
                                  start=(kb == 0),
                                  stop=(kb == nkb - 1),
                              )

                      for kb in range(nkb):
                          i = kb - 4 * qt
                          lo = max(i, 0) * P
                          psS = psmm.tile([P, 2, QTILE], F32, tag="mm",
                                          name="psS")
                          for h in range(2):
                              nc.tensor.matmul(
                                  psS[:, h, lo:QTILE],
                                  kt_dr[:, 2 * h : 2 * h + 2, ts(kb, P)],
                                  qt_dr[:, 2 * h : 2 * h + 2,
                                        qt * QTILE + lo : (qt + 1) * QTILE],
                                  perf_mode=mybir.MatmulPerfMode.DoubleRow,
                              )
                          pt = pt_pool.tile([P, 2, QTILE], BF16, tag="pt",
                                            name="pt")
                          nc.scalar.activation(
                              pt[:, :, lo:QTILE],
                              psS[:, :, lo:QTILE],
                              ActF.Exp,
                              scale=float(SCALE),
                          )
                          if i >= 0:
                              # zero the upper triangle of the diagonal
                              # 128-col strip (bf16, SBUF: DVE 2x mode)
                              nc.vector.tensor_tensor(
                                  pt[:, :, lo : lo + P],
                                  pt[:, :, lo : lo + P],
                                  trimask[:, None, :].broadcast_to((P, 2, P)),
                                  op=Alu.mult,
                              )
                          pts[kb] = pt
                          if kb == 1:
                              # this q-tile's V chain + the previous
                              # q-tile's finalize land here, behind the
                              # first two queued S matmuls, so the PE
                              # never parks them ahead of the exp stream
                              own.drain_until(("v", 4 * qt + 3))
                              flush_pending()
                          if kb >= 2:
                              o_step(kb - 2)
                          if kb == 3 and qt + 1 < NQT:
                              # prefetch the next q-tile's Q/K projection +
                              # DR shuffles so their DMAs clear the queue
                              # before the exp stream needs them
                              own.drain_until(("qk", qt + 1))
                          pop_steps(work, 2, b * NQT + qt)
                      o_step(nkb - 2)
                      o_step(nkb - 1)
                      oT = osb_pool.tile([HD + 1, 2, QTILE], BF16,
                                         tag="oT")
                      if final:
                          # tail: pipeline per head — head 0's transpose,
                          # normalize and writeback overlap head 1's
                          # eviction
                          fin = fin_pool.tile([P, 4, 2, HD], F32,
                                              tag="fin")
                          for h in range(2):
                              nc.vector.tensor_copy(
                                  oT[:, h, :], psO[: HD + 1, h, :]
                              )
                              finalize_head(b, qt, oT[:, h, :], h, fin,
                                            True)
                      else:
                          # evict O'^T per head (frees each psO bank as
                          # soon as its accumulation ends), defer the
                          # transpose/normalize into the next q-tile
                          for h in range(2):
                              nc.vector.tensor_copy(
                                  oT[:, h, :], psO[: HD + 1, h, :]
                              )
                          pending.append((b, qt, oT))

              # ---------- schedule: one continuous pipeline ----------
              # PE warmup on constants: keeps the tensor engine streaming
              # (and its clock ramping) while the HWDGE works through the
              # first x chunks' descriptors.
              warm = const.tile([P, QTILE], BF16, tag="warm")
              nc.vector.memset(warm[:], 0.5)
              psW = pstr.tile([P, QTILE], F32, tag="tr", name="psW")
              for _ in range(6):
                  nc.tensor.matmul(psW[:], warm[:, :P], warm[:])

              # Pool-queue load order for the cold start:
              # x8q0, w8, bias, x8q1, wv, then (from the gen) drshuf-k0 —
              # whose sem wait head-blocks the queue — then the xt chunks.
              gen0 = prep_setup(0)
              gen0.pop(1, 0)  # x8(b0) quarter 0
              nc.gpsimd.dma_start(
                  w8qk[:], w8_d[:].rearrange("n p dt j m -> p n dt j m")
              )
              nc.gpsimd.dma_start(bias3[:], b_d[:].rearrange("n p -> p n"))
              gen0.pop(1, 0)  # alloc_vp
              nc.gpsimd.dma_start(
                  w_sb["v"][:],
                  w_d["v"][:].rearrange("(cb p) m -> p cb m", p=P),
              )
              gen1 = prep_setup(1)
              work = [gen0, gen1]
              attention(0, gen0, work)
              attention(1, gen1, work)
              flush_pending()
              flush_pending()

    nc.compile()
    return nc


_CACHED = {}


def _to_f8(x: np.ndarray) -> np.ndarray:
    import ml_dtypes

    return np.ascontiguousarray(
        np.asarray(x, np.float32).astype(ml_dtypes.float8_e4m3)
    )


def _build_in_maps(inputs):
    x = np.ascontiguousarray(np.asarray(inputs["x"], np.float32))
    # host-side prep: transpose x to [B, H, T], cast matmul operands to bf16
    xT = x.transpose(0, 2, 1)
    xt = _to_bf16(xT)
    # fp8 DoubleRow copy of x for the Q/K projections:
    # [b, p, dt, j, t] = x^T[b, 256j + 128dt + p, t]
    x8 = _to_f8(
        np.asarray(xT, np.float32)
        .reshape(B, 4, 2, P, T)
        .transpose(0, 3, 2, 1, 4)
    )
    Wq, Wk, Wv = inputs["Wq"], inputs["Wk"], inputs["Wv"]
    bq, bk, bv = inputs["bq"], inputs["bk"], inputs["bv"]

    in_maps = []
    for m in range(NCORES):
        sl = slice(m * P, (m + 1) * P)  # 128 output channels = 2 heads
        def w8fmt(W):
            # [p, dt, j, m] = W.T[256j + 128dt + p, _DR_PERM[m]]
            return _to_f8(
                np.asarray(W, np.float32)[sl, :].T[:, _DR_PERM]
                .reshape(4, 2, P, P)
                .transpose(2, 1, 0, 3)
            )

        in_maps.append({
            "xt": xt,
            "x8": x8,
            "w8qk": np.ascontiguousarray(
                np.stack([w8fmt(Wq), w8fmt(Wk)])),
            "wvt": _to_bf16(np.asarray(Wv)[sl, :].T),
            "bqkv": np.ascontiguousarray(np.stack([
                np.asarray(bq, np.float32)[sl][_DR_PERM],
                np.asarray(bk, np.float32)[sl][_DR_PERM],
                np.asarray(bv, np.float32)[sl],
            ])),
        })
    return in_maps


def kernel(x, Wq, bq, Wk, bk, Wv, bv):
    from concourse.bass_utils import run_bass_kernel_spmd

    if "nc" not in _CACHED:
        _CACHED["nc"] = _build_program()
    nc = _CACHED["nc"]

    in_maps = _build_in_maps(
        dict(x=x, Wq=Wq, bq=bq, Wk=Wk, bk=bk, Wv=Wv, bv=bv)
    )

    res = run_bass_kernel_spmd(nc, in_maps, core_ids=list(range(NCORES)))
    out = np.concatenate(
        [res.results[m]["out"] for m in range(NCORES)], axis=-1
    )
    return out


# revision 81
# speedup vs baseline: 1.0432x; 1.0432x over previous
"""Multi-head masked self-attention on 8 trn2 NeuronCores.

Problem: B=2, T=2048, H=1024, nH=16 heads (head_dim=64), causal softmax
attention with QKV projections; scores scaled by 1/sqrt(H).

Sharding: heads across cores (2 heads per core), both batches on every core
(B*nH = 32 (b,h) pairs -> 4 per core). QKV weights column-sharded by head:
core m gets W[128m:128m+128, :] of each projection matrix.

The ACT engine (exp eviction of the score stream, 0.833 ns/elem, no fast
mode) is the roofline: ~36 us of exp per batch. The schedule keeps ACT as
gapless as possible, and keeps DMA/instruction counts low for the
hardware's (unmodeled-in-sim) per-descriptor and per-ldweights costs:
  - DMA issue (descriptor gen ~1.2-1.9 us of queue time each) is spread
    over the SP and Pool queues: bulk loads metered through the Pool
    (SWDGE) queue in priority order, DR shuffles and output writebacks on
    SP, the cold-start K shuffle on the idle ACT queue.
  - xt (V-path) loads move in 256-column halves — the narrowest width
    with full DMA bandwidth — so a bulk load never holds the serial DMA
    engines for more than ~1.5 us in front of a critical shuffle copy.
  - Q/K prep runs one projection tile ahead of the V chain, and each
    q-tile's V chain is drained at k-block 1 of its own k-loop, behind
    the first two queued S matmuls, so the PE never parks prep work in
    front of the exp stream.

Attention per q-tile of 512 (4 q-tiles):
  S^T[k, q] = K^T.T(128-slice) @ Q^T -> PSUM f32 [128, 2, 512] (two banks,
      one per head), fp8 DoubleRow, ACT evicts exp(S/32) -> bf16 (no
      row-max: |S/32| << 1 for this distribution), multiplicative bf16
      triangle mask on diagonal blocks (DVE).
  O'^T [65, 512] += V'[kb].T @ P^T[kb], one PSUM bank per head (row 64
      accumulates the softmax denominator Z from the same quantized P),
      trailing the score stream by two k-blocks.
  Finalize is software-pipelined into the NEXT q-tile's k-loop: per head a
      DVE eviction [65, 512] -> SBUF, 4 PE transposes into one PSUM bank,
      one DVE reciprocal, one broadcasted DVE multiply; one merged output
      DMA per q-tile (both heads) on the SP queue. The last q-tile
      finalizes inline with per-head writebacks to shorten the tail.

PSUM budget (8 banks): psS 2x2, psO 1+1, shared ring 2 (projection psA
[128,512] / V' transposes / finalize transposes).
"""
import sys

sys.path.insert(0, "/opt/trn_rl_repo")

import numpy as np

B = 2
T = 2048
H = 1024
NHEADS = 16
HD = 64
NCORES = 8
HEADS_PER_CORE = NHEADS // NCORES  # 2
P = 128
CB = H // P            # 8 contraction blocks for projections
QTILE = 512
NQT = T // QTILE       # 4 q-tiles
NTT = T // QTILE       # 4 projection column tiles
NKB = T // P           # 16 k-blocks
SCALE = 1.0 / np.sqrt(np.float32(H))  # 1/32


def _to_bf16(x: np.ndarray) -> np.ndarray:
    import ml_dtypes

    return np.ascontiguousarray(
        np.asarray(x, np.float32).astype(ml_dtypes.bfloat16)
    )


# DoubleRow channel permutation for the Q/K projections: on-chip channel
# pi feeds DR-layout slot [p, hd] = [pi // 4, pi % 4] via a linear-order
# SBUF->SBUF DMA, and slot [p, (h, dt)] must hold logical head channel
# d = 64h + 32dt + p of the core's 128-channel slice.
_DR_PERM = np.array(
    [64 * ((pi % 4) // 2) + 32 * (pi % 2) + pi // 4 for pi in range(128)]
)


def _build_program(reps: int = 1):
    import contextlib
    import concourse.tile as tile
    from concourse import bacc, mybir
    from concourse.masks import make_identity
    from concourse.bass import ts

    F32 = mybir.dt.float32
    BF16 = mybir.dt.bfloat16
    F8 = mybir.dt.float8e4
    ActF = mybir.ActivationFunctionType
    Alu = mybir.AluOpType

    nc = bacc.Bacc("TRN2", target_bir_lowering=False, debug=False)

    xt_d = nc.dram_tensor("xt", [B, H, T], BF16, kind="ExternalInput")
    x8_d = nc.dram_tensor("x8", [B, P, 2, 4, T], F8, kind="ExternalInput")
    w_d = {"v": nc.dram_tensor("wvt", [H, P], BF16,
                               kind="ExternalInput")}
    w8_d = nc.dram_tensor("w8qk", [2, P, 2, 4, P], F8,
                          kind="ExternalInput")
    b_d = nc.dram_tensor("bqkv", [3, P], F32, kind="ExternalInput")
    out_d = nc.dram_tensor("out", [B, T, P], F32, kind="ExternalOutput")

    with tile.TileContext(nc) as tc:
        with (
            tc.tile_pool(name="const", bufs=1) as const,
            tc.tile_pool(name="xt", bufs=2) as xt_pool,
            tc.tile_pool(name="qkv", bufs=2) as qkv_pool,
            tc.tile_pool(name="qkdr", bufs=2) as qkdr_pool,
            tc.tile_pool(name="vp", bufs=4) as vp_pool,
            tc.tile_pool(name="pt", bufs=5) as pt_pool,
            tc.tile_pool(name="osb", bufs=4) as osb_pool,
            tc.tile_pool(name="fin", bufs=6) as fin_pool,
            tc.tile_pool(name="psmm", bufs=2, space="PSUM") as psmm,
            tc.tile_pool(name="pso", bufs=1, space="PSUM") as pso,
            tc.tile_pool(name="pstr", bufs=2, space="PSUM") as pstr,
        ):
            # ---- constants ----
            ident = const.tile([P, P], BF16, tag="ident")
            make_identity(nc, ident[:])
            # multiplicative causal triangle for the diagonal 128x128 block
            # of a k-block: keep (1) where k_local <= q_local else 0; applied
            # to P after the exp so the mask is off the S->exp critical path
            trimask = const.tile([P, P], BF16, tag="trimask")
            nc.gpsimd.memset(trimask[:], 1.0)
            nc.gpsimd.affine_select(
                out=trimask[:],
                in_=trimask[:],
                compare_op=mybir.AluOpType.is_ge,
                fill=0.0,
                base=0,
                pattern=[[1, P]],
                channel_multiplier=-1,
            )
            ones16 = const.tile([P, NKB], BF16, tag="ones16")
            nc.vector.memset(ones16[:], 1.0)
            # scratch target for the cold-start load-gating reads
            gate_scr = const.tile([1, 4], BF16, tag="gate_scr")
            nc.vector.memset(gate_scr[:], 0.0)

            w_sb = {"v": const.tile([P, CB, P], BF16, tag="wv", name="wv")}
            w8qk = const.tile([P, 2, 2, 4, P], F8, tag="w8qk", name="w8qk")
            w8_sb = {"q": w8qk[:, 0], "k": w8qk[:, 1]}
            bias3 = const.tile([P, 3], F32, tag="b3", name="b3")
            bias_sb = {n: bias3[:, i : i + 1] for i, n in enumerate("qkv")}

            rep_ctx = (
                tc.For_i(0, reps, 1,
                         hint_engines=(mybir.EngineType.PE,
                                       mybir.EngineType.Activation,
                                       mybir.EngineType.DVE,
                                       mybir.EngineType.SP))
                if reps > 1 else contextlib.nullcontext()
            )
            with rep_ctx:
              # ---------- prep stream ----------
              class Prep:
                  """Generator of ('step', fn) / ('pace', n) /
                  (marker-kind, idx) items with marker memory, so drains
                  are idempotent. ('pace', n) items hold opportunistic
                  pops until global attention progress reaches n, so a
                  batch's projection work is never emitted into the PE
                  queue before its loads can plausibly have landed."""

                  def __init__(self, gen):
                      self.gen = gen
                      self.seen = set()
                      self.pending = None
                      self.done = False

                  def _next(self):
                      if self.pending is not None:
                          it, self.pending = self.pending, None
                          return it
                      it = next(self.gen, None)
                      if it is None:
                          self.done = True
                      return it

                  def drain_until(self, marker):
                      while marker not in self.seen and not self.done:
                          it = self._next()
                          if it is None:
                              return
                          kind, val = it
                          if kind == "step":
                              val()
                          elif kind != "pace":
                              self.seen.add((kind, val))

                  def pop(self, budget, pace):
                      while budget > 0 and not self.done:
                          it = self._next()
                          if it is None:
                              return
                          kind, val = it
                          if kind == "step":
                              val()
                              budget -= 1
                          elif kind == "pace":
                              if val > pace:
                                  self.pending = it
                                  return
                          else:
                              self.seen.add((kind, val))

              def pop_steps(preps, budget, pace):
                  for p in preps:
                      if not p.done:
                          p.pop(budget, pace)
                          return

              def prep_setup(b):
                  """Projection + V'-build for batch b as a stream of small
                  step closures with ('qk', tt) / ('v', kb) markers.

                  DMA queue assignment (descriptor-gen cost ~1.2-1.7 us
                  serializes per queue): x8 quarters + Q-shuffles on SP;
                  weights, K-shuffles and xt chunks on Pool, ordered so the
                  shuffle copies hit the (serial) DMA engines before the
                  bulk xt loads."""
                  qt_sb = qkv_pool.tile([P, T], F8, tag="qt", name="qt_sb")
                  kt_sb = qkv_pool.tile([P, T], F8, tag="kt", name="kt_sb")
                  vt_sb = qkv_pool.tile([P, T], BF16, tag="vt", name="vt_sb")
                  dsts = {"q": qt_sb, "k": kt_sb, "v": vt_sb}
                  qt_dr = qkdr_pool.tile([32, 4, T], F8, tag="qdr",
                                         name="qt_dr")
                  kt_dr = qkdr_pool.tile([32, 4, T], F8, tag="kdr",
                                         name="kt_dr")
                  drs = {"q": qt_dr, "k": kt_dr}
                  state = {}
                  vprime = []
                  xt = xt_pool.tile([P, CB, T], BF16, tag="xt", name="xt_sb")
                  x8 = xt_pool.tile([P, 2, 4, T], F8, tag="x8", name="x8_sb")

                  def load_x8(c, eng=None):
                      def run():
                          (eng or nc.gpsimd).dma_start(
                              x8[:, :, :, ts(c, T // 4)],
                              x8_d[b, :, :, :, ts(c, T // 4)],
                          )
                      return run

                  def load_xt(c0, w):
                      """cols [c0*128, (c0+w)*128), all cb, on Pool."""
                      def run():
                          nc.gpsimd.dma_start(
                              xt[:, :, c0 * P : (c0 + w) * P],
                              xt_d[b, :, c0 * P : (c0 + w) * P]
                              .rearrange("(cb p) t -> p cb t", p=P),
                          )
                      return run

                  prep_out[b] = (qt_dr, kt_dr, vprime)

                  def alloc_psa():
                      state["psA"] = pstr.tile([P, QTILE], F32, tag="tr",
                                               name="psA")

                  def mms(n, tt):
                      def run():
                          # fp8 DoubleRow: contraction c = 256j+128dt+p,
                          # 4 column-pair passes of 256 each
                          for j in range(4):
                              nc.tensor.matmul(
                                  state["psA"][:],
                                  w8_sb[n][:, :, j, :],
                                  x8[:, :, j, ts(tt, QTILE)],
                                  start=(j == 0),
                                  stop=(j == 3),
                                  perf_mode=mybir.MatmulPerfMode.DoubleRow,
                              )
                      return run

                  def evict(n, tt):
                      def run():
                          nc.vector.tensor_scalar_add(
                              dsts[n][:, ts(tt, QTILE)],
                              state["psA"][:],
                              bias_sb[n],
                          )
                      return run

                  def mms_v(c0, w, half):
                      """V projection for cols [c0*128, (c0+w)*128), cb
                      blocks [4*half, 4*half+4)."""
                      def run():
                          for cb in range(4 * half, 4 * half + 4):
                              nc.tensor.matmul(
                                  state["psA"][:, : w * P],
                                  w_sb["v"][:, cb, :],
                                  xt[:, cb, c0 * P : (c0 + w) * P],
                                  start=(cb == 0),
                                  stop=(cb == CB - 1),
                              )
                      return run

                  def evict_v(c0, w):
                      def run():
                          nc.vector.tensor_scalar_add(
                              vt_sb[:, c0 * P : (c0 + w) * P],
                              state["psA"][:, : w * P],
                              bias_sb["v"],
                          )
                      return run

                  def alloc_vp():
                      vp2 = vp_pool.tile([P, NKB, 2, HD + 1], BF16, tag="vp",
                                         name="vp2")
                      nc.vector.tensor_copy(
                          vp2[:, :, :, HD],
                          ones16[:, :, None].broadcast_to((P, NKB, 2)),
                      )
                      vprime.append(vp2)

                  def vtr(kb):
                      def run():
                          trp = pstr.tile([P, P], BF16, tag="tr", name="trp")
                          nc.tensor.transpose(
                              trp[:], vt_sb[:, ts(kb, P)], ident[:],
                          )
                          nc.vector.tensor_copy(
                              vprime[0][:, kb, :, :HD],
                              trp[:].rearrange("p (h d) -> p h d", h=2),
                          )
                      return run

                  def drshuf(n, tt):
                      # cold start only: K's shuffle goes out on the idle
                      # ACT queue so it doesn't serialize behind Q's on
                      # the SP sequencer (held through the sem wait)
                      eng = (nc.scalar if (b == 0 and tt == 0 and n == "k")
                             else nc.sync)
                      def run():
                          eng.dma_start(
                              drs[n][:, :, ts(tt, QTILE)],
                              dsts[n][:, ts(tt, QTILE)],
                          )
                      return run

                  def cold_gate():
                      """DVE sliver reads (in-order, data-dep'd on the
                      two tile-0 DR shuffles) that touch the x8 quarter-1
                      and xt second-half regions: the Tile WAR deps make
                      those bulk loads wait for the shuffle copies, so
                      they can never jump ahead of them on the serial
                      DMA engines."""
                      def run():
                          nc.vector.tensor_copy(
                              gate_scr[:, 0:1], kt_dr[0:1, 0:1, 0:1])
                          nc.vector.tensor_copy(
                              gate_scr[:, 1:2], qt_dr[0:1, 0:1, 0:1])
                          nc.vector.tensor_copy(
                              gate_scr[:, 3:4], xt[0:1, 0:1, 2 * P : 2 * P + 1])
                      return run

                  def v_half(c):
                      """V chain for k-blocks c, c+1 (256 columns: the
                      narrowest width that keeps full DMA bandwidth), so
                      V'[kb] lands just in time for the trailing O
                      accumulation. The xt load is yielded separately by
                      the caller to control DMA-queue order."""
                      yield ("step", alloc_psa)
                      yield ("step", mms_v(c, 2, 0))
                      yield ("step", mms_v(c, 2, 1))
                      yield ("step", evict_v(c, 2))
                      for kb in range(c, c + 2):
                          yield ("step", vtr(kb))
                          yield ("v", kb)

                  def qk_block(tt):
                      for n in "qk":
                          yield ("step", alloc_psa)
                          yield ("step", mms(n, tt))
                          yield ("step", evict(n, tt))
                          yield ("step", drshuf(n, tt))
                      yield ("qk", tt)

                  def v_tile(tt):
                      """V chain for tile tt: two half loads (a full
                      1 MB quarter would hog the serial DMA engines for
                      ~3 us and starve the DR-shuffle copies), one
                      full-width projection, four transposes."""
                      yield ("step", load_xt(4 * tt, 2))
                      yield ("step", load_xt(4 * tt + 2, 2))
                      yield ("step", alloc_psa)
                      yield ("step", mms_v(4 * tt, 4, 0))
                      yield ("step", mms_v(4 * tt, 4, 1))
                      yield ("step", evict_v(4 * tt, 4))
                      for kb in range(4 * tt, 4 * tt + 4):
                          yield ("step", vtr(kb))
                          yield ("v", kb)

                  def gen():
                      # Q/K prep runs one tile ahead of the V chain: the
                      # exp stream is gated by Q/K only, and V'[kb] is
                      # needed one k-block behind the exp of the same
                      # q-tile. x8 quarter 0 of batch 0 on SP (the
                      # cold-start critical path); everything else is
                      # metered through the Pool descriptor-gen queue.
                      # Batch 1's pace marks hold its projection work
                      # until batch 0's attention has progressed far
                      # enough for batch 1's loads to have landed.
                      yield ("step", load_x8(0, eng=nc.sync if b == 0
                                             else None))
                      yield ("step", alloc_vp)
                      for it in qk_block(0):
                          yield it
                      yield ("step", load_xt(0, 2))
                      if b == 0:
                          yield ("step", cold_gate())
                      yield ("step", load_x8(1))
                      for it in v_half(0):
                          yield it
                      for it in qk_block(1):
                          yield it
                      yield ("step", load_xt(2, 2))
                      for it in v_half(2):
                          yield it
                      for tt in range(2, NTT):
                          yield ("step", load_x8(tt))
                          for it in qk_block(tt):
                              yield it
                          for it in v_tile(tt - 1):
                              yield it
                      for it in v_tile(NTT - 1):
                          yield it

                  return Prep(gen())

              prep_out = {}

              # deferred finalize state: list of (b, qt, fin, filled-count)
              pending = []

              def finalize_head(fb, fqt, oT_h, h, fin, split):
                  """Transpose + normalize one head of a finished q-tile;
                  with split=True, also write it back on its own DMA."""
                  trp4 = pstr.tile([P, 4, P], BF16, tag="tr", name="trp4")
                  for j in range(4):
                      nc.tensor.transpose(
                          trp4[:, j, : HD + 1],
                          oT_h[:, ts(j, P)],
                          ident[: HD + 1, : HD + 1],
                      )
                  rec = fin_pool.tile([P, 4], F32, tag="rec")
                  nc.vector.reciprocal(rec[:], trp4[:, :, HD])
                  nc.vector.tensor_tensor(
                      fin[:, :, h, :],
                      trp4[:, :, :HD],
                      rec[:, :, None].broadcast_to((P, 4, HD)),
                      op=Alu.mult,
                  )
                  if split:
                      nc.sync.dma_start(
                          out_d[fb, ts(fqt, QTILE), ts(h, HD)]
                          .rearrange("(j p) c -> p j c", p=P),
                          fin[:, :, h, :],
                      )

              def flush_pending():
                  if not pending:
                      return
                  fb, fqt, oT = pending.pop()
                  fin = fin_pool.tile([P, 4, 2, HD], F32, tag="fin")
                  for h in range(2):
                      finalize_head(fb, fqt, oT[:, h, :], h, fin, False)
                  # one merged output DMA per q-tile (both heads), on the
                  # SP queue (the Pool queue meters the bulk loads)
                  nc.sync.dma_start(
                      out_d[fb, ts(fqt, QTILE), :].rearrange(
                          "(j p) c -> p j c", p=P
                      ),
                      fin[:].rearrange("p j h d -> p j (h d)"),
                  )

              def attention(b, own, work):
                  """own: this batch's prep stream (('qk', qt)-gated at each
                  q-tile start, ('v', kb)-gated before each trailing O
                  step); work: prep streams drained opportunistically, two
                  steps per k-block.

                  The P@V' accumulation trails the score stream by one
                  k-block so the PE never waits on the exp: the k-loop body
                  issues S(kb), exp(kb), then O(kb-1)."""
                  qt_dr, kt_dr, vprime = prep_out[b]
                  for qt in range(NQT):
                      own.drain_until(("qk", qt))
                      psO = pso.tile([P, 2, QTILE], F32, tag="o",
                                     name="psO")
                      nkb = 4 * qt + 4
                      pts = {}
                      final = b == B - 1 and qt == NQT - 1

                      def o_step(kb):
                          lo = max(kb - 4 * qt, 0) * P
                          pt = pts.pop(kb)
                          for h in range(2):
                              nc.tensor.matmul(
                                  psO[: HD + 1, h, lo:QTILE],
                                  vprime[0][:, kb, h, :],
                                  pt[:, h, lo:QTILE],
                                  start=(kb == 0),
                                  stop=(kb == nkb - 1),
                              )

                      for kb in range(nkb):
                          i = kb - 4 * qt
                          lo = max(i, 0) * P
                          psS = psmm.tile([P, 2, QTILE], F32, tag="mm",
                                          name="psS")
                          for h in range(2):
                              nc.tensor.matmul(
                                  psS[:, h, lo:QTILE],
                                  kt_dr[:, 2 * h : 2 * h + 2, ts(kb, P)],
                                  qt_dr[:, 2 * h : 2 * h + 2,
                                        qt * QTILE + lo : (qt + 1) * QTILE],
                                  perf_mode=mybir.MatmulPerfMode.DoubleRow,
                              )
                          pt = pt_pool.tile([P, 2, QTILE], BF16, tag="pt",
                                            name="pt")
                          nc.scalar.activation(
                              pt[:, :, lo:QTILE],
                              psS[:, :, lo:QTILE],
                              ActF.Exp,
                              scale=float(SCALE),
                          )
                          if i >= 0:
                              # zero the upper triangle of the diagonal
                              # 128-col strip (bf16, SBUF: DVE 2x mode)
                              nc.vector.tensor_tensor(
                                  pt[:, :, lo : lo + P],
                                  pt[:, :, lo : lo + P],
                                  trimask[:, None, :].broadcast_to((P, 2, P)),
                                  op=Alu.mult,
                              )
                          pts[kb] = pt
                          if kb == 1:
                              # this q-tile's V chain + the previous
                              # q-tile's finalize land here, behind the
                              # first two queued S matmuls, so the PE
                              # never parks them ahead of the exp stream
                              # (q-tile 0: only the first half — the
                              # second still waits on the gated loads)
                              own.drain_until(
                                  ("v", 4 * qt + (1 if qt == 0 else 3)))
                              flush_pending()
                          if kb >= 2:
                              o_step(kb - 2)
                          if kb == 3 and qt + 1 < NQT:
                              # prefetch the next q-tile's Q/K projection +
                              # DR shuffles so their DMAs clear the queue
                              # before the exp stream needs them
                              own.drain_until(("qk", qt + 1))
                          pop_steps(work, 2, b * NQT + qt)
                          if b == 0 and qt >= 3 and len(work) > 1 \
                                  and not work[1].done:
                              # pull batch 1's loads/prep into batch 0's
                              # ACT-heavy late q-tiles (PE slack) so the
                              # batch handoff doesn't bottleneck on PE
                              work[1].pop(2, 0)
                      own.drain_until(("v", nkb - 1))
                      o_step(nkb - 2)
                      o_step(nkb - 1)
                      oT = osb_pool.tile([HD + 1, 2, QTILE], BF16,
                                         tag="oT")
                      if final:
                          # tail: pipeline per head — head 0's transpose,
                          # normalize and writeback overlap head 1's
                          # eviction
                          fin = fin_pool.tile([P, 4, 2, HD], F32,
                                              tag="fin")
                          for h in range(2):
                              nc.vector.tensor_copy(
                                  oT[:, h, :], psO[: HD + 1, h, :]
                              )
                              finalize_head(b, qt, oT[:, h, :], h, fin,
                                            True)
                      else:
                          # evict O'^T per head (frees each psO bank as
                          # soon as its accumulation ends), defer the
                          # transpose/normalize into the next q-tile
                          for h in range(2):
                              nc.vector.tensor_copy(
                                  oT[:, h, :], psO[: HD + 1, h, :]
                              )
                          pending.append((b, qt, oT))

              # ---------- schedule: one continuous pipeline ----------
              # PE warmup on constants: keeps the tensor engine streaming
              # (and its clock ramping) while the HWDGE works through the
              # first x chunks' descriptors.
              warm = const.tile([P, QTILE], BF16, tag="warm")
              nc.vector.memset(warm[:], 0.5)
              psW = pstr.tile([P, QTILE], F32, tag="tr", name="psW")
              for _ in range(13):
                  nc.tensor.matmul(psW[:], warm[:, :P], warm[:])

              # Pool-queue load order for the cold start:
              # x8q0, w8, bias, x8q1, wv, then (from the gen) drshuf-k0 —
              # whose sem wait head-blocks the queue — then the xt chunks.
              gen0 = prep_setup(0)
              gen0.pop(1, 0)  # x8(b0) quarter 0
              nc.gpsimd.dma_start(
                  w8qk[:], w8_d[:].rearrange("n p dt j m -> p n dt j m")
              )
              nc.gpsimd.dma_start(bias3[:], b_d[:].rearrange("n p -> p n"))
              gen0.pop(1, 0)  # alloc_vp
              nc.gpsimd.dma_start(
                  w_sb["v"][:],
                  w_d["v"][:].rearrange("(cb p) m -> p cb m", p=P),
              )
              gen1 = prep_setup(1)
              work = [gen0, gen1]
              attention(0, gen0, work)
              attention(1, gen1, work)
              flush_pending()
              flush_pending()

    nc.compile()
    return nc


_CACHED = {}


def _to_f8(x: np.ndarray) -> np.ndarray:
    import ml_dtypes

    return np.ascontiguousarray(
        np.asarray(x, np.float32).astype(ml_dtypes.float8_e4m3)
    )


def _build_in_maps(inputs):
    x = np.ascontiguousarray(np.asarray(inputs["x"], np.float32))
    # host-side prep: transpose x to [B, H, T], cast matmul operands to bf16
    xT = x.transpose(0, 2, 1)
    xt = _to_bf16(xT)
    # fp8 DoubleRow copy of x for the Q/K projections:
    # [b, p, dt, j, t] = x^T[b, 256j + 128dt + p, t]
    x8 = _to_f8(
        np.asarray(xT, np.float32)
        .reshape(B, 4, 2, P, T)
        .transpose(0, 3, 2, 1, 4)
    )
    Wq, Wk, Wv = inputs["Wq"], inputs["Wk"], inputs["Wv"]
    bq, bk, bv = inputs["bq"], inputs["bk"], inputs["bv"]

    in_maps = []
    for m in range(NCORES):
        sl = slice(m * P, (m + 1) * P)  # 128 output channels = 2 heads
        def w8fmt(W):
            # [p, dt, j, m] = W.T[256j + 128dt + p, _DR_PERM[m]]
            return _to_f8(
                np.asarray(W, np.float32)[sl, :].T[:, _DR_PERM]
                .reshape(4, 2, P, P)
                .transpose(2, 1, 0, 3)
            )

        in_maps.append({
            "xt": xt,
            "x8": x8,
            "w8qk": np.ascontiguousarray(
                np.stack([w8fmt(Wq), w8fmt(Wk)])),
            "wvt": _to_bf16(np.asarray(Wv)[sl, :].T),
            "bqkv": np.ascontiguousarray(np.stack([
                np.asarray(bq, np.float32)[sl][_DR_PERM],
                np.asarray(bk, np.float32)[sl][_DR_PERM],
                np.asarray(bv, np.float32)[sl],
            ])),
        })
    return in_maps


def kernel(x, Wq, bq, Wk, bk, Wv, bv):
    from concourse.bass_utils import run_bass_kernel_spmd

    if "nc" not in _CACHED:
        _CACHED["nc"] = _build_program()
    nc = _CACHED["nc"]

    in_maps = _build_in_maps(
        dict(x=x, Wq=Wq, bq=bq, Wk=Wk, bk=bk, Wv=Wv, bv=bv)
    )

    res = run_bass_kernel_spmd(nc, in_maps, core_ids=list(range(NCORES)))
    out = np.concatenate(
        [res.results[m]["out"] for m in range(NCORES)], axis=-1
    )
    return out
